# revision 28
# baseline (speedup 1.0000x reference)
"""Trainium2 Bass kernel for nn_AttentionBasedMerger.

Reference computation (per batch element b, SQ=1):
  q = input @ Wq + bq                      -> (NH, HD)  [tiny]
  k = retrieval @ Wk + bk                  -> (SK, NH, HD)
  v = retrieval @ Wv + bv                  -> (SK, NH, HD)
  scores[h,j] = cos_sim(q[h], k[j,h])
  p = (scores+1)/2 ; 2-way gumbel-softmax gate with external uniform noise
  probs[h,j] = gate[...,0]
  ctx[h] = sum_j probs[h,j] v[j,h]         -> (NH, HD)
  out = ctx.flat @ Wd + bd                 -> (HID,)

Algebraic restructuring used here (exact up to fp reassociation):
  - v-projection is never materialized:
      ctx[h] = (sum_j probs[h,j] x[j]) @ Wv_h + (sum_j probs[h,j]) * bv_h
    so only a (NH x SK) @ (SK x HID) GEMM ("m"-matmul) plus a tiny
    per-head (HID x HD) GEMM is needed -- 64x fewer FLOPs than full v.
  - scores come from the k-projection run once:
      s_raw[j,h]  = x[j] @ (Wk @ qhat_blockdiag)  (extra 16 psum columns)
      kbk[j,h]    = x[j] @ wbk                    (bias correction columns)
      ssq[j,h]    = sum_d k0[j,h,d]^2             (squared-eviction + reduce)
      ||k||^2     = ssq + 2*kbk + sum(bk_h^2)
      scores      = (s_raw + qhat.bk_h) * rsqrt(||k||^2)
  - the 2-way gumbel softmax collapses to a stable rational:
      probs = p*A1 / (p*A1 + (1-p)*A0),  A_i = EPS - log(u_i + EPS)
    (A0/A1 are pure elementwise transforms of the noise input, applied on
    the host during input staging).

Sharding: pure data-parallel over batch, 8 batch elements per core.
"""

import os
import sys

sys.path.insert(0, "/opt/trn_rl_repo")

import numpy as np

import concourse.bass as bass
import concourse.tile as tile
from concourse import bacc, mybir
from concourse.bass_utils import run_bass_kernel_spmd
from concourse.masks import make_identity

F32 = mybir.dt.float32
BF16 = mybir.dt.bfloat16
AX = mybir.AxisListType
OP = mybir.AluOpType
AF = mybir.ActivationFunctionType

B, SK, HID, NH, HD = 64, 2048, 1024, 16, 64
NCORES = 8
BL = B // NCORES  # 8 batch elems per core
CI = HID // 128  # 8 contraction chunks
JC = SK // 128  # 16 seq chunks
JG = 2  # seq chunks per xT load group
EPS = 1e-20

# matmul precision mode: "f32" (native fp32, 4 cyc/row),
# "f32r" (relaxed fp32, 1 cyc/row, precision TBD on HW),
# "hilo" (bf16 hi+lo 3-pass, ~fp32 precision, 3 cyc/row)
MM_MODE = os.environ.get("MM_MODE", "f32")


def _r(ap, dtype):
    """bitcast an AP when running in f32r mode (no-op otherwise)."""
    if dtype is None:
        return ap
    return ap.bitcast(dtype)


def build_nc(mode=MM_MODE):
    nc = bacc.Bacc("TRN2", target_bir_lowering=False, debug=False, num_devices=NCORES)

    dram = {}

    def din(name, shape, dt=F32):
        dram[name] = nc.dram_tensor(name, list(shape), dt, kind="ExternalInput").ap()
        return dram[name]

    xn = din("xn", [BL, SK, HID])  # natural retrieval slice
    u_a0 = din("a0", [BL, SK, NH])  # EPS - log(u0 + EPS)
    u_a1 = din("a1", [BL, SK, NH])
    inT = din("inT", [HID, BL])  # input_tensor slice, transposed
    wq = din("wq", [HID, HID])
    wv = din("wv", [HID, HID])
    wd = din("wd", [HID, HID])
    bq = din("bq", [1, HID])
    bk = din("bk", [1, HID])
    bv = din("bv", [1, HID])
    bd = din("bd", [1, HID])
    sbrep = din("sbrep", [1, NH * JC])  # tile(sum(bk_h^2), JC)
    wbk = din("wbk", [HID, NH])
    wkT = din("wkT", [HID, HID])  # Wk transposed (for wq_eff = Wk @ qhat_blk)
    if mode == "hilo":
        xt_h = din("xt_h", [BL, HID, SK], BF16)
        xt_l = din("xt_l", [BL, HID, SK], BF16)
        wk_h = din("wk_h", [HID, HID], BF16)
        wk_l = din("wk_l", [HID, HID], BF16)
        xn_h = din("xn_h", [BL, SK, HID], BF16)
        xn_l = din("xn_l", [BL, SK, HID], BF16)
        wk = None
        xt = None
    else:
        xt = din("xt", [BL, HID, SK])  # retrieval slice, transposed
        wk = din("wk", [HID, HID])

    out = nc.dram_tensor("o", [BL, HID], F32, kind="ExternalOutput").ap()

    # f32r: moving+stationary operands bitcast to float32r
    rdt = mybir.dt.float32r if mode == "f32r" else None

    with tile.TileContext(nc) as tc:
        with (
            tc.tile_pool(name="const", bufs=1) as constp,
            tc.tile_pool(name="wkp", bufs=1) as wkp,
            tc.tile_pool(name="bigw", bufs=2) as bigwp,
            tc.tile_pool(name="xtg", bufs=2) as xtgp,
            tc.tile_pool(name="xnp", bufs=3) as xnp,
            tc.tile_pool(name="ksqp", bufs=2) as ksqp,
            tc.tile_pool(name="gate", bufs=2) as gatep,
            tc.tile_pool(name="probs", bufs=BL) as probsp,
            tc.tile_pool(name="anoise", bufs=4) as ap_pool,
            tc.tile_pool(name="psum", bufs=2, space="PSUM") as pp,
            tc.tile_pool(name="psum_small", bufs=3, space="PSUM") as pps,
        ):
            # ---------------- constants ----------------
            ident = constp.tile([128, 128], F32, tag="ident")
            make_identity(nc, ident[:])
            ones = constp.tile([128, 1], F32, tag="ones")
            nc.vector.memset(ones[:], 1.0)
            if mode == "hilo":
                ones16 = constp.tile([128, 1], BF16, tag="ones16")
                nc.vector.memset(ones16[:], 1.0)

            def load_bcast(name, ap_in, parts, free):
                t = constp.tile([parts, free], F32, tag=name)
                nc.sync.dma_start(t[:], ap_in.to_broadcast((parts, free)))
                return t

            bq8 = load_bcast("bq8", bq, BL, HID)
            bk8 = load_bcast("bk8", bk, BL, HID)
            bv8 = load_bcast("bv8", bv, BL, HID)
            bd8 = load_bcast("bd8", bd, BL, HID)
            sb_sb = load_bcast("sb_sb", sbrep, 128, NH * JC)

            inT_sb = constp.tile([128, CI, BL], F32, tag="inT")
            nc.sync.dma_start(
                inT_sb[:], inT.rearrange("(ci p) b -> p ci b", p=128)
            )

            wbk_sb = constp.tile([128, CI, NH], F32, tag="wbk")
            nc.sync.dma_start(wbk_sb[:], wbk.rearrange("(ci p) h -> p ci h", p=128))
            if mode == "hilo":
                wk_sb_h = wkp.tile([128, CI, HID], BF16, tag="wk_h")
                nc.sync.dma_start(
                    wk_sb_h[:], wk_h.rearrange("(ci p) f -> p ci f", p=128)
                )
                wk_sb_l = wkp.tile([128, CI, HID], BF16, tag="wk_l")
                nc.sync.dma_start(
                    wk_sb_l[:], wk_l.rearrange("(ci p) f -> p ci f", p=128)
                )
                wk_sb = None
            else:
                wk_sb = wkp.tile([128, CI, HID], F32, tag="wk")
                nc.sync.dma_start(wk_sb[:], wk.rearrange("(ci p) f -> p ci f", p=128))

            # ---------------- q projection (all local b at once) -------------
            # q[b, f] = input[b] @ Wq + bq ; normalize per head; build the
            # block-diagonal qhat matrix feeding the s_raw psum columns.
            pq = pp.tile([128, 1024], F32, tag="big")
            for hf in range(2):
                wq_sb = bigwp.tile([128, CI, 512], F32, tag="bigw")
                nc.sync.dma_start(
                    wq_sb[:],
                    wq.rearrange("(ci p) f -> p ci f", p=128)[
                        :, :, hf * 512 : (hf + 1) * 512
                    ],
                )
                for ci in range(CI):
                    nc.tensor.matmul(
                        pq[0:BL, hf * 512 : (hf + 1) * 512],
                        inT_sb[:, ci, :],
                        wq_sb[:, ci, :],
                        start=(ci == 0),
                        stop=(ci == CI - 1),
                    )
            q_sb = constp.tile([BL, HID], F32, tag="q_sb")
            nc.vector.tensor_add(q_sb[:], pq[0:BL, :], bq8[:])
            tmp8 = constp.tile([BL, HID], F32, tag="tmp8")
            nc.scalar.activation(tmp8[:], q_sb[:], AF.Square)
            qssq = constp.tile([BL, NH], F32, tag="qssq")
            nc.vector.reduce_sum(
                qssq[:], tmp8[:].rearrange("b (h d) -> b h d", d=HD), axis=AX.X
            )
            # rqn = 1/sqrt(qssq), one Newton refinement
            rq = constp.tile([BL, NH], F32, tag="rq")
            nc.scalar.activation(rq[:], qssq[:], AF.Sqrt)
            nc.vector.reciprocal(rq[:], rq[:])
            tq = constp.tile([BL, NH], F32, tag="tq")
            nc.vector.tensor_mul(tq[:], rq[:], rq[:])
            nc.vector.tensor_mul(tq[:], tq[:], qssq[:])
            nc.vector.tensor_scalar(tq[:], tq[:], -0.5, 1.5, OP.mult, OP.add)
            nc.vector.tensor_mul(rq[:], rq[:], tq[:])
            # qhat = q * rqn (broadcast rqn over head dim)
            qn = constp.tile([BL, HID], F32, tag="qn")
            nc.vector.tensor_mul(
                qn[:].rearrange("b (h d) -> b h d", d=HD),
                q_sb[:].rearrange("b (h d) -> b h d", d=HD),
                rq[:].unsqueeze(2).to_broadcast([BL, NH, HD]),
            )
            # cqn[b,h] = dot(bk_h, qhat_h)
            nc.vector.tensor_mul(tmp8[:], qn[:], bk8[:])
            cqn = constp.tile([BL, NH], F32, tag="cqn")
            nc.vector.reduce_sum(
                cqn[:], tmp8[:].rearrange("b (h d) -> b h d", d=HD), axis=AX.X
            )
            # broadcast each b's cqn row across 128 partitions (via DRAM bounce)
            with tc.tile_pool(name="dramtmp", bufs=1, space="DRAM") as dramp:
                cqn_dram = dramp.tile([BL, NH], F32, tag="cqn_dram")
                nc.sync.dma_start(cqn_dram[:], cqn[:])
                cqn_bc = []
                for b in range(BL):
                    t = constp.tile([128, NH], F32, tag=f"cqn_bc{b}")
                    nc.sync.dma_start(
                        t[:], cqn_dram[b : b + 1, :].to_broadcast((128, NH))
                    )
                    cqn_bc.append(t)

            # transpose qhat -> [c, b] and assemble block-diagonal Qblk
            qnT = constp.tile([128, CI, BL], F32, tag="qnT")
            for ci in range(CI):
                ptr = pps.tile([128, 128], F32, tag="small")
                nc.tensor.transpose(
                    ptr[:, 0:BL],
                    qn[:, ci * 128 : (ci + 1) * 128],
                    ident[0:BL, 0:BL],
                )
                nc.vector.tensor_copy(qnT[:, ci, :], ptr[:, 0:BL])
            # qnblk[f, fi, b, h] = block-diagonal qhat (rows = Wk output feats)
            qnblk = constp.tile([128, CI, BL, NH], F32, tag="qnblk")
            nc.vector.memset(qnblk[:], 0.0)
            for fi in range(CI):
                for b in range(BL):
                    for half in range(2):
                        h = 2 * fi + half
                        nc.vector.tensor_copy(
                            qnblk[
                                half * 64 : (half + 1) * 64,
                                fi : fi + 1,
                                b : b + 1,
                                h : h + 1,
                            ],
                            qnT[half * 64 : (half + 1) * 64, fi : fi + 1, b : b + 1],
                        )
            # wq_eff[c, (b,h)] = sum_f Wk[c,f] * qnblk[f, (b,h)] via PE with WkT,
            # packed together with wbk into the s-matmul moving operand swblk.
            swblk = constp.tile([128, CI, BL, 32], F32, tag="swblk")
            for half in range(2):
                wkT_sb = bigwp.tile([128, CI, 512], F32, tag="bigw", name="wkT_sb")
                nc.sync.dma_start(
                    wkT_sb[:],
                    wkT.rearrange("(fi p) c -> p fi c", p=128)[
                        :, :, half * 512 : (half + 1) * 512
                    ],
                )
                for cc in range(4):
                    ci = half * 4 + cc
                    pwq = pps.tile([128, 128], F32, tag="small")
                    for fi in range(CI):
                        nc.tensor.matmul(
                            pwq[:, :],
                            wkT_sb[:, fi, cc * 128 : (cc + 1) * 128],
                            qnblk[:, fi, :, :],
                            start=(fi == 0),
                            stop=(fi == CI - 1),
                        )
                    nc.vector.tensor_copy(
                        swblk[:, ci : ci + 1, :, 0:16],
                        pwq[:].rearrange("p (b h) -> p b h", h=NH).unsqueeze(1),
                    )
            for ci in range(CI):
                nc.vector.tensor_copy(
                    swblk[:, ci : ci + 1, :, 16:32],
                    wbk_sb[:, ci : ci + 1, :]
                    .unsqueeze(2)
                    .to_broadcast([128, 1, BL, NH]),
                )
            if mode == "hilo":
                swblk_h = constp.tile([128, CI, BL, 32], BF16, tag="swblk_h")
                nc.vector.tensor_copy(swblk_h[:], swblk[:])
                swblk_l = constp.tile([128, CI, BL, 32], BF16, tag="swblk_l")
                nc.vector.tensor_sub(swblk_l[:], swblk[:], swblk_h[:])

            # ---------------- k projection + gate, per local batch ----------
            probs_all = []
            for b in range(BL):
                ssq_all = gatep.tile([128, JC, NH], F32, tag="ssq")
                sk_all = gatep.tile([128, JC, 32], F32, tag="sk")
                for jg in range(JC // JG):
                    if mode == "hilo":
                        xg_h = xtgp.tile([128, CI, JG * 128], BF16, tag="xg_h")
                        nc.sync.dma_start(
                            xg_h[:],
                            xt_h[b].rearrange("(ci p) j -> p ci j", p=128)[
                                :, :, jg * JG * 128 : (jg + 1) * JG * 128
                            ],
                        )
                        xg_l = xtgp.tile([128, CI, JG * 128], BF16, tag="xg_l")
                        nc.sync.dma_start(
                            xg_l[:],
                            xt_l[b].rearrange("(ci p) j -> p ci j", p=128)[
                                :, :, jg * JG * 128 : (jg + 1) * JG * 128
                            ],
                        )
                    else:
                        xg = xtgp.tile([128, CI, JG * 128], F32, tag="xg")
                        nc.sync.dma_start(
                            xg[:],
                            xt[b].rearrange("(ci p) j -> p ci j", p=128)[
                                :, :, jg * JG * 128 : (jg + 1) * JG * 128
                            ],
                        )
                    for jl in range(JG):
                        jc = jg * JG + jl
                        jsl = slice(jl * 128, (jl + 1) * 128)
                        pk = pp.tile([128, 1024], F32, tag="big")
                        ps = pps.tile([128, 128], F32, tag="small")
                        for ci in range(CI):
                            st = ci == 0
                            sp = ci == CI - 1
                            if mode == "hilo":
                                lh = xg_h[:, ci, jsl]
                                ll = xg_l[:, ci, jsl]
                                for bank in range(2):
                                    fs = slice(bank * 512, (bank + 1) * 512)
                                    nc.tensor.matmul(
                                        pk[:, fs], lh, wk_sb_h[:, ci, fs],
                                        start=st, stop=False,
                                    )
                                    nc.tensor.matmul(
                                        pk[:, fs], lh, wk_sb_l[:, ci, fs],
                                        start=False, stop=False,
                                    )
                                    nc.tensor.matmul(
                                        pk[:, fs], ll, wk_sb_h[:, ci, fs],
                                        start=False, stop=sp,
                                    )
                                nc.tensor.matmul(
                                    ps[:, 0:32], lh, swblk_h[:, ci, b, :],
                                    start=st, stop=False,
                                )
                                nc.tensor.matmul(
                                    ps[:, 0:32], lh, swblk_l[:, ci, b, :],
                                    start=False, stop=False,
                                )
                                nc.tensor.matmul(
                                    ps[:, 0:32], ll, swblk_h[:, ci, b, :],
                                    start=False, stop=sp,
                                )
                            else:
                                lhs = _r(xg[:, ci, jsl], rdt)
                                for bank in range(2):
                                    fs = slice(bank * 512, (bank + 1) * 512)
                                    nc.tensor.matmul(
                                        pk[:, fs],
                                        lhs,
                                        _r(wk_sb[:, ci, fs], rdt),
                                        start=st,
                                        stop=sp,
                                    )
                                nc.tensor.matmul(
                                    ps[:, 0:32],
                                    lhs,
                                    _r(swblk[:, ci, b, :], rdt),
                                    start=st,
                                    stop=sp,
                                )
                        # evictions: k^2 via ACT square; segmented reduce on DVE
                        ksq = ksqp.tile([128, HID], F32, tag="ksq")
                        nc.scalar.activation(ksq[:], pk[:, :], AF.Square)
                        nc.vector.reduce_sum(
                            ssq_all[:, jc, :],
                            ksq[:].rearrange("p (h d) -> p h d", d=HD),
                            axis=AX.X,
                        )
                        nc.vector.tensor_copy(sk_all[:, jc, :], ps[:, 0:32])

                # ---------------- gate (rational gumbel softmax) ----------
                a0_t = ap_pool.tile([128, JC, NH], F32, tag="a0")
                nc.sync.dma_start(
                    a0_t[:], u_a0[b].rearrange("(jc p) h -> p jc h", p=128)
                )
                a1_t = ap_pool.tile([128, JC, NH], F32, tag="a1")
                nc.sync.dma_start(
                    a1_t[:], u_a1[b].rearrange("(jc p) h -> p jc h", p=128)
                )

                # buffer-reusing gate math: g1..g3 are scratch [128, JC, NH]
                g1 = gatep.tile([128, JC, NH], F32, tag="g1")  # ssq2 -> qt -> den
                g2 = gatep.tile([128, JC, NH], F32, tag="g2")  # r -> num
                g3 = gatep.tile([128, JC, NH], F32, tag="g3")  # newton tmp / sc / rd
                nc.vector.scalar_tensor_tensor(
                    g1[:], sk_all[:, :, 16:32], 2.0, ssq_all[:], OP.mult, OP.add
                )
                nc.vector.tensor_add(
                    g1[:], g1[:], sb_sb[:].rearrange("p (jc h) -> p jc h", h=NH)
                )
                # g2 = rsqrt(g1) with one Newton step
                nc.scalar.activation(g2[:], g1[:], AF.Sqrt)
                nc.vector.reciprocal(g2[:], g2[:])
                nc.vector.tensor_mul(g3[:], g2[:], g2[:])
                nc.vector.tensor_mul(g3[:], g3[:], g1[:])
                nc.vector.tensor_scalar(g3[:], g3[:], -0.5, 1.5, OP.mult, OP.add)
                nc.vector.tensor_mul(g2[:], g2[:], g3[:])
                # g3 = scores = (s_raw + cqn) * rsqrt
                nc.vector.tensor_add(
                    g3[:],
                    sk_all[:, :, 0:16],
                    cqn_bc[b][:].unsqueeze(1).to_broadcast([128, JC, NH]),
                )
                nc.vector.tensor_mul(g3[:], g3[:], g2[:])
                # p = (scores+1)/2 ; num = p*A1 ; den = num + (1-p)*A0
                nc.vector.tensor_scalar(g2[:], g3[:], 0.5, 0.5, OP.mult, OP.add)
                nc.vector.tensor_scalar(g1[:], g3[:], -0.5, 0.5, OP.mult, OP.add)
                nc.vector.tensor_mul(g2[:], g2[:], a1_t[:])  # num
                nc.vector.tensor_mul(g1[:], g1[:], a0_t[:])
                nc.vector.tensor_add(g1[:], g1[:], g2[:])  # den
                # probs = num * refined_recip(den)
                nc.vector.reciprocal(g3[:], g1[:])
                nc.vector.tensor_mul(g1[:], g1[:], g3[:])
                nc.vector.tensor_scalar(g1[:], g1[:], -1.0, 2.0, OP.mult, OP.add)
                nc.vector.tensor_mul(g3[:], g3[:], g1[:])
                if mode == "hilo":
                    probs = gatep.tile([128, JC, NH], F32, tag="probs_t")
                    nc.vector.tensor_mul(probs[:], g2[:], g3[:])
                    ph = probsp.tile([128, JC, NH], BF16, tag="probs_h")
                    nc.vector.tensor_copy(ph[:], probs[:])
                    pl = probsp.tile([128, JC, NH], BF16, tag="probs_l")
                    nc.vector.tensor_sub(pl[:], probs[:], ph[:])
                    probs_all.append((ph, pl))
                else:
                    probs = probsp.tile([128, JC, NH], F32, tag="probs")
                    nc.vector.tensor_mul(probs[:], g2[:], g3[:])
                    probs_all.append(probs)

            # ---------------- m-matmul: m[b] = probs[b].T @ x[b] ------------
            # each b's [NH, HID] block lives at a 32-aligned partition slot
            # (engine ops only accept base partitions {0,32,64,96})
            m_tiles = [
                constp.tile([128, HID], F32, tag="m_allA", name="m_allA"),
                constp.tile([128, HID], F32, tag="m_allB", name="m_allB"),
            ]
            psp = pps.tile([128, 128], F32, tag="small")
            for b in range(BL):
                pm = pp.tile([128, 1024], F32, tag="big")
                for jc in range(JC):
                    if mode == "hilo":
                        xnt_h = xnp.tile([128, HID], BF16, tag="xn_h")
                        nc.sync.dma_start(
                            xnt_h[:], xn_h[b, jc * 128 : (jc + 1) * 128, :]
                        )
                        xnt_l = xnp.tile([128, HID], BF16, tag="xn_l")
                        nc.sync.dma_start(
                            xnt_l[:], xn_l[b, jc * 128 : (jc + 1) * 128, :]
                        )
                        ph, pl = probs_all[b]
                        st = jc == 0
                        sp = jc == JC - 1
                        for bank in range(2):
                            fs = slice(bank * 512, (bank + 1) * 512)
                            nc.tensor.matmul(
                                pm[0:NH, fs], ph[:, jc, :], xnt_h[:, fs],
                                start=st, stop=False,
                            )
                            nc.tensor.matmul(
                                pm[0:NH, fs], ph[:, jc, :], xnt_l[:, fs],
                                start=False, stop=False,
                            )
                            nc.tensor.matmul(
                                pm[0:NH, fs], pl[:, jc, :], xnt_h[:, fs],
                                start=False, stop=sp,
                            )
                        # sp via exact hi+lo accumulation (one psum group)
                        nc.tensor.matmul(
                            psp[0:NH, b : b + 1], ph[:, jc, :], ones16[:],
                            start=(jc == 0), stop=False,
                        )
                        nc.tensor.matmul(
                            psp[0:NH, b : b + 1], pl[:, jc, :], ones16[:],
                            start=False, stop=(jc == JC - 1),
                        )
                    else:
                        xnt = xnp.tile([128, HID], F32, tag="xn")
                        nc.sync.dma_start(
                            xnt[:], xn[b, jc * 128 : (jc + 1) * 128, :]
                        )
                        probs = probs_all[b]
                        for bank in range(2):
                            fs = slice(bank * 512, (bank + 1) * 512)
                            nc.tensor.matmul(
                                pm[0:NH, fs],
                                _r(probs[:, jc, :], rdt),
                                _r(xnt[:, fs], rdt),
                                start=(jc == 0),
                                stop=(jc == JC - 1),
                            )
                        # sp[b,h] = sum_j probs
                        nc.tensor.matmul(
                            psp[0:NH, b : b + 1],
                            probs[:, jc, :],
                            ones[:],
                            start=(jc == 0),
                            stop=(jc == JC - 1),
                        )
                slot = (b % 4) * 32
                nc.vector.tensor_copy(
                    m_tiles[b // 4][slot : slot + NH, :], pm[0:NH, :]
                )

            # ---------------- ctx + final dense ------------------------------
            # transpose m -> mT[c, (b,h)]
            mT = constp.tile([128, CI, 128], F32, tag="mT")
            for ci in range(CI):
                for b in range(BL):
                    slot = (b % 4) * 32
                    ptr = pps.tile([128, 128], F32, tag="small")
                    nc.tensor.transpose(
                        ptr[:, 0:NH],
                        m_tiles[b // 4][
                            slot : slot + NH, ci * 128 : (ci + 1) * 128
                        ],
                        ident[slot : slot + NH, slot : slot + NH],
                        tile_position=(slot, 0),
                    )
                    nc.vector.tensor_copy(
                        mT[:, ci : ci + 1, b * NH : (b + 1) * NH],
                        ptr[:, 0:NH].unsqueeze(1),
                    )
            # sp: psum [NH, BL] -> sbuf -> transpose -> [BL, NH]
            spT = constp.tile([NH, BL], F32, tag="spT")
            nc.vector.tensor_copy(spT[:], psp[0:NH, 0:BL])
            psp2 = pps.tile([128, 128], F32, tag="small")
            nc.tensor.transpose(psp2[0:BL, 0:NH], spT[:], ident[0:NH, 0:NH])
            sp_all = constp.tile([BL, NH], F32, tag="sp_all")
            nc.vector.tensor_copy(sp_all[:], psp2[0:BL, 0:NH])

            # ctx[b, (h,d)] = sum_ci mT[:, ci, (b,h)] @ Wv[ci, (h,d)]
            pctx = pp.tile([128, 1024], F32, tag="big")
            for hf in range(2):
                wv_sb = bigwp.tile([128, CI, 512], F32, tag="bigw")
                nc.sync.dma_start(
                    wv_sb[:],
                    wv.rearrange("(ci p) f -> p ci f", p=128)[
                        :, :, hf * 512 : (hf + 1) * 512
                    ],
                )
                for hh in range(NH // 2):
                    h = hf * (NH // 2) + hh
                    for ci in range(CI):
                        nc.tensor.matmul(
                            pctx[0:BL, h * HD : (h + 1) * HD],
                            mT[:, ci, h : 128 : NH],
                            wv_sb[:, ci, hh * HD : (hh + 1) * HD],
                            start=(ci == 0),
                            stop=(ci == CI - 1),
                        )
            # ctx += sp * bv (broadcast over d)
            ctx_sb = constp.tile([BL, HID], F32, tag="ctx")
            tbv = constp.tile([BL, HID], F32, tag="tbv")
            nc.vector.tensor_mul(
                tbv[:].rearrange("b (h d) -> b h d", d=HD),
                bv8[:].rearrange("b (h d) -> b h d", d=HD),
                sp_all[:].unsqueeze(2).to_broadcast([BL, NH, HD]),
            )
            nc.vector.tensor_add(ctx_sb[:], pctx[0:BL, :], tbv[:])
            # transpose ctx -> [c, b]
            ctxT = constp.tile([128, CI, BL], F32, tag="ctxT")
            for ci in range(CI):
                ptr = pps.tile([128, 128], F32, tag="small")
                nc.tensor.transpose(
                    ptr[:, 0:BL],
                    ctx_sb[:, ci * 128 : (ci + 1) * 128],
                    ident[0:BL, 0:BL],
                )
                nc.vector.tensor_copy(ctxT[:, ci, :], ptr[:, 0:BL])
            # out = ctx @ Wd + bd
            po = pp.tile([128, 1024], F32, tag="big")
            for hf in range(2):
                wd_sb = bigwp.tile([128, CI, 512], F32, tag="bigw")
                nc.sync.dma_start(
                    wd_sb[:],
                    wd.rearrange("(ci p) f -> p ci f", p=128)[
                        :, :, hf * 512 : (hf + 1) * 512
                    ],
                )
                for ci in range(CI):
                    nc.tensor.matmul(
                        po[0:BL, hf * 512 : (hf + 1) * 512],
                        ctxT[:, ci, :],
                        wd_sb[:, ci, :],
                        start=(ci == 0),
                        stop=(ci == CI - 1),
                    )
            o_sb = constp.tile([BL, HID], F32, tag="o_sb")
            nc.vector.tensor_add(o_sb[:], po[0:BL, :], bd8[:])
            nc.sync.dma_start(out[:], o_sb[:])

    nc.compile()
    return nc


def _split_hilo(x):
    import ml_dtypes

    h = x.astype(ml_dtypes.bfloat16)
    l = (x - h.astype(np.float32)).astype(ml_dtypes.bfloat16)
    return h, l


def prep_in_maps(inputs, mode=MM_MODE):
    """Host-side staging: shard over batch, transpose/relayout, noise logs."""
    it = np.asarray(inputs["input_tensor"], dtype=np.float32)  # (B, 1, HID)
    rt = np.asarray(inputs["retrieval_tensor"], dtype=np.float32)  # (B, SK, HID)
    un = np.asarray(inputs["u_noise"], dtype=np.float32)  # (B, NH, 1, SK, 2)
    wq = np.asarray(inputs["Wq"], dtype=np.float32)
    wk = np.asarray(inputs["Wk"], dtype=np.float32)
    wv = np.asarray(inputs["Wv"], dtype=np.float32)
    wd = np.asarray(inputs["Wd"], dtype=np.float32)
    bq = np.asarray(inputs["bq"], dtype=np.float32).reshape(1, HID)
    bk = np.asarray(inputs["bk"], dtype=np.float32).reshape(1, HID)
    bv = np.asarray(inputs["bv"], dtype=np.float32).reshape(1, HID)
    bd = np.asarray(inputs["bd"], dtype=np.float32).reshape(1, HID)

    bk_heads = bk.reshape(NH, HD)
    wbk = np.einsum(
        "chd,hd->ch", wk.reshape(HID, NH, HD), bk_heads
    ).astype(np.float32)  # (HID, NH)
    sb = np.tile((bk_heads**2).sum(axis=1), JC).reshape(1, NH * JC).astype(np.float32)

    # A_i = EPS - log(u_i + EPS), computed in fp32 like the reference
    u0 = un[:, :, 0, :, 0].transpose(0, 2, 1)  # (B, SK, NH)
    u1 = un[:, :, 0, :, 1].transpose(0, 2, 1)
    a0 = (np.float32(EPS) - np.log(u0 + np.float32(EPS), dtype=np.float32)).astype(
        np.float32
    )
    a1 = (np.float32(EPS) - np.log(u1 + np.float32(EPS), dtype=np.float32)).astype(
        np.float32
    )

    shared = {
        "wq": wq, "wv": wv, "wd": wd,
        "bq": bq, "bk": bk, "bv": bv, "bd": bd,
        "sbrep": sb,
    }
    shared["wbk"] = wbk
    shared["wkT"] = np.ascontiguousarray(wk.T)
    if mode == "hilo":
        wk_h, wk_l = _split_hilo(wk)
        shared.update(wk_h=wk_h, wk_l=wk_l)
    else:
        shared.update(wk=wk)

    in_maps = []
    for c in range(NCORES):
        bs = slice(c * BL, (c + 1) * BL)
        xn_c = np.ascontiguousarray(rt[bs])
        xt_c = np.ascontiguousarray(rt[bs].transpose(0, 2, 1))
        m = {
            "xn": xn_c,
            "a0": np.ascontiguousarray(a0[bs]),
            "a1": np.ascontiguousarray(a1[bs]),
            "inT": np.ascontiguousarray(it[bs, 0, :].T),
            **shared,
        }
        if mode == "hilo":
            m["xt_h"], m["xt_l"] = _split_hilo(xt_c)
            m["xn_h"], m["xn_l"] = _split_hilo(xn_c)
            del m["xn"]
        else:
            m["xt"] = xt_c
        in_maps.append(m)
    return in_maps


_NC_CACHE = {}


def kernel(**inputs) -> np.ndarray:
    mode = MM_MODE
    if mode not in _NC_CACHE:
        _NC_CACHE[mode] = build_nc(mode)
    nc = _NC_CACHE[mode]
    in_maps = prep_in_maps(inputs, mode)
    res = run_bass_kernel_spmd(nc, in_maps, core_ids=list(range(NCORES)))
    return np.concatenate([res.results[c]["o"] for c in range(NCORES)], axis=0)
